# revision 30
# baseline (speedup 1.0000x reference)
"""Trainium2 Bass kernel for Performer-style (FAVOR+) causal linear attention.

Reference computation (per batch b=1, heads h=16, seq s=2048, d=64, r=64):
  qh = split_heads((q @ wq + bq) * d^-0.25)     kh likewise, vh = split_heads(v @ wv + bv)
  q' = (1/sqrt(d)) * exp(qh @ wg - 0.5*||qh||^2)   k' likewise
  attn[s] = (q'_s . sum_{j<=s} k'_j v_j^T) / (eps + q'_s . sum_{j<=s} k'_j)
  out = merge_heads(attn) @ wc + bc

Sharding: 2 heads per core (16 heads over 8 cores). Each core receives the
full (transposed, fp16) q/k/v plus its 128-column slice of the projection
weights, computes its heads' attention via a chunked causal scan (chunk=128),
projects through its 128-row slice of wc, and returns a (2048, 1024) fp16
partial. The host sums the 8 partials and adds the output bias.
"""

import sys

if "/opt/trn_rl_repo" not in sys.path:
    sys.path.insert(0, "/opt/trn_rl_repo")

import math
from contextlib import ExitStack

import numpy as np

D_MODEL = 1024
N_HEADS = 16
D = 64  # head depth
R = 64  # kernel features
S = 2048
N_CORES = 8
HPC = N_HEADS // N_CORES  # heads per core = 2
CW = HPC * D  # per-core channel width = 128
P = 128
ST = 512  # projection s-tile width
NST = S // ST  # 4
C = 128  # scan chunk
NCH = S // C  # 16
KT = D_MODEL // P  # 8 contraction tiles
NORM_D = float(D ** (-0.25))
LN_RSQRT_D = float(-0.5 * math.log(D))  # exp(x + this) = exp(x)/sqrt(d)

_CACHE = {}


def _build_bass(nst=NST, nch=NCH, stage=9):
    import concourse.bass as bass
    import concourse.mybir as mybir
    import concourse.tile as tile
    from concourse.bacc import Bacc

    f16 = mybir.dt.float16
    f32 = mybir.dt.float32
    AF = mybir.ActivationFunctionType
    Alu = mybir.AluOpType

    nc = Bacc(trn_type="TRN2")

    qT = nc.dram_tensor("qT", [D_MODEL, S], f16, kind="ExternalInput")
    kT = nc.dram_tensor("kT", [D_MODEL, S], f16, kind="ExternalInput")
    vT = nc.dram_tensor("vT", [D_MODEL, S], f16, kind="ExternalInput")
    wq = nc.dram_tensor("wq", [D_MODEL, CW], f16, kind="ExternalInput")
    wk = nc.dram_tensor("wk", [D_MODEL, CW], f16, kind="ExternalInput")
    wv = nc.dram_tensor("wv", [D_MODEL, CW], f16, kind="ExternalInput")
    # aux: [ident(128) | mask(128) | wg2(64) | ng2(64)] packed along free dim
    aux = nc.dram_tensor("aux", [P, 2 * P + 2 * R], f16, kind="ExternalInput")
    bqkv = nc.dram_tensor("bqkv", [CW, 3], f32, kind="ExternalInput")
    wc = nc.dram_tensor("wc", [CW, D_MODEL], f16, kind="ExternalInput")
    out = nc.dram_tensor("out", [S, D_MODEL], f16, kind="ExternalOutput")

    with tile.TileContext(nc) as tc, ExitStack() as ctx:
        # ---- constant / weight tiles ----
        const = ctx.enter_context(tc.tile_pool(name="const", bufs=1))
        w_sb = {}
        for name, drt in (("wq", wq), ("wk", wk), ("wv", wv)):
            t = const.tile([P, KT * CW], f16, tag=name, name=f"wt_{name}")
            # dest[p, k*CW + c] <- w[k*P + p, c]
            dst = t[:].rearrange("p (k c) -> p k c", k=KT)
            sr = drt[:, :].rearrange("(k p) c -> p k c", p=P)
            nc.sync.dma_start(dst, sr)
            for k in range(KT):
                w_sb[(name, k)] = t[:, k * CW : (k + 1) * CW]
        aux_sb = const.tile([P, 2 * P + 2 * R], f16, tag="aux")
        nc.sync.dma_start(aux_sb[:], aux[:, :])
        id_sb = aux_sb[:, 0:P]
        mask_sb = aux_sb[:, P : 2 * P]
        wg_sb = aux_sb[:, 2 * P : 2 * P + R]
        ng_sb = aux_sb[:, 2 * P + R : 2 * P + 2 * R]
        wc_sb = const.tile([CW, D_MODEL], f16, tag="wc")
        nc.sync.dma_start(wc_sb[:], wc[:, :])
        b_all = const.tile([CW, 3], f32, tag="ball")
        nc.sync.dma_start(b_all[:], bqkv[:, :])
        b_sb = {"bq": b_all[:, 0:1], "bk": b_all[:, 1:2], "bv": b_all[:, 2:3]}
        ebias = const.tile([P, 1], f32, tag="ebias")
        nc.vector.memset(ebias[:], LN_RSQRT_D)

        # ---- pools ----
        xin = ctx.enter_context(tc.tile_pool(name="xin", bufs=24))
        tmp_pool = ctx.enter_context(tc.tile_pool(name="tmp", bufs=2))
        big_psum = ctx.enter_context(tc.tile_pool(name="bigp", bufs=2, space="PSUM"))
        prj_psum = big_psum
        phi_psum = big_psum
        qp_pool = ctx.enter_context(tc.tile_pool(name="qp", bufs=NST))
        kp_pool = ctx.enter_context(tc.tile_pool(name="kp", bufs=NST))
        vh_pool = ctx.enter_context(tc.tile_pool(name="vh", bufs=NST))

        # stream inputs: one DMA per (tensor, k-tile, s-half); first halves first
        x_sb = {}
        for name, srct in (("q", qT), ("k", kT), ("v", vT)):
            for k in range(KT):
                x_sb[(name, k)] = xin.tile([P, S], f16, tag="xin", name=f"x_{name}{k}")
        H = S // 2
        for half in range(2):
            for name, srct in (("q", qT), ("k", kT), ("v", vT)):
                for k in range(KT):
                    nc.sync.dma_start(
                        x_sb[(name, k)][:, half * H : (half + 1) * H],
                        srct[k * P : (k + 1) * P, half * H : (half + 1) * H],
                    )

        # per s-tile: projections for q, k, v + feature maps for q, k
        qp_t, kp_t, vh_t = [], [], []

        def emit_stile(st):
            sl = slice(st * ST, (st + 1) * ST)
            for name in ("q", "k", "v"):
                pp = prj_psum.tile([P, ST], f32, tag="big", name=f"prj_{st}_{name}")
                for k in range(KT):
                    nc.tensor.matmul(
                        pp[:], w_sb[("w" + name, k)][:], x_sb[(name, k)][:, sl],
                        start=(k == 0), stop=(k == KT - 1)
                    )
                if name == "v":
                    vh = vh_pool.tile([P, ST], f16, tag="vh")
                    # vh = psum + bv
                    nc.vector.tensor_scalar(vh[:], pp[:], b_sb["bv"][:], None, Alu.add)
                    vh_t.append(vh)
                else:
                    # tmp = psum * NORM_D + b  (b pre-scaled by NORM_D on host)
                    tmp = tmp_pool.tile([P, ST], f16, tag="tmpl")
                    nc.vector.tensor_scalar(
                        tmp[:], pp[:], NORM_D, b_sb["b" + name][:], Alu.mult, Alu.add
                    )
                    tmp2 = tmp_pool.tile([P, ST], f16, tag="tmps")
                    nc.vector.tensor_tensor(tmp2[:], tmp[:], tmp[:], Alu.mult)
                    fp = phi_psum.tile([P, ST], f32, tag="big", name=f"phi_{st}_{name}")
                    nc.tensor.matmul(fp[0:D, :], wg_sb[0:D, :], tmp[0:D, :], start=True, stop=False)
                    nc.tensor.matmul(fp[0:D, :], ng_sb[0:D, :], tmp2[0:D, :], start=False, stop=True)
                    nc.tensor.matmul(
                        fp[D:P, :], wg_sb[D:P, :], tmp[D:P, :],
                        start=True, stop=False, tile_position=(D, D),
                    )
                    nc.tensor.matmul(
                        fp[D:P, :], ng_sb[D:P, :], tmp2[D:P, :],
                        start=False, stop=True, tile_position=(D, D),
                    )
                    dst_pool = qp_pool if name == "q" else kp_pool
                    pt = dst_pool.tile([P, ST], f16, tag="qkp")
                    nc.scalar.activation(pt[:], fp[:], AF.Exp, bias=ebias[:])
                    (qp_t if name == "q" else kp_t).append(pt)

        # ---- attention scan (chunk = 128) ----
        tp_psum = ctx.enter_context(tc.tile_pool(name="tpp", bufs=2, space="PSUM"))
        at_psum = ctx.enter_context(tc.tile_pool(name="atp", bufs=1, space="PSUM"))
        o_psum = ctx.enter_context(tc.tile_pool(name="op", bufs=1, space="PSUM"))
        s_psum = ctx.enter_context(tc.tile_pool(name="sp", bufs=1, space="PSUM"))
        ot_psum = tp_psum
        f_psum = ctx.enter_context(tc.tile_pool(name="fpp", bufs=1, space="PSUM"))
        sc_pool = ctx.enter_context(tc.tile_pool(name="sc", bufs=6))
        ot_pool = ctx.enter_context(tc.tile_pool(name="ot", bufs=8))
        out_pool = ctx.enter_context(tc.tile_pool(name="outp", bufs=16))

        s_ps = s_psum.tile([P, D + 1], f32, tag="S")
        # persistent V_aug tiles (even/odd) with ones columns at 64 and 129
        vaug = []
        s_sb = []
        for par in range(2):
            va = const.tile([P, 2 * (D + 1)], f16, tag=f"vaug{par}")
            ones_ap = va[:].rearrange("p (b c) -> p b c", c=D + 1)[:, :, D]
            nc.vector.memset(ones_ap, 1.0)
            vaug.append(va)
            s_sb.append(const.tile([P, D + 1], f16, tag=f"ssb{par}", name=f"ssb{par}"))

        def emit_chunk(c):
            if stage < 2:
                return
            st, off = c // 4, (c % 4) * C
            csl = slice(off, off + C)
            va = vaug[c % 2]
            # K' and V transposed to s-major via PE transpose
            ktp = tp_psum.tile([P, P], f16, tag="tp")
            nc.tensor.transpose(ktp[:], kp_t[st][:, csl], id_sb[:])
            ks = sc_pool.tile([P, P], f16, tag="ks")
            nc.vector.tensor_copy(ks[:], ktp[:])
            vtp = tp_psum.tile([P, P], f16, tag="tp")
            nc.tensor.transpose(vtp[:], vh_t[st][:, csl], id_sb[:])
            va_dst = va[:].rearrange("p (b c) -> p b c", c=D + 1)[:, :, 0:D]
            nc.scalar.activation(va_dst, vtp[:].rearrange("p (b c) -> p b c", c=D), AF.Copy)

            if stage < 3:
                return
            # intra-chunk attention AT[j,i] per head (row-packed pair)
            atm = []
            for h in range(HPC):
                atp = at_psum.tile([P, P], f32, tag="at", name=f"at{h}_{c}")
                nc.tensor.matmul(
                    atp[:], kp_t[st][h * D : (h + 1) * D, csl],
                    qp_t[st][h * D : (h + 1) * D, csl],
                    tile_position=(h * D, 0), start=True, stop=True,
                )
                am = sc_pool.tile([P, P], f16, tag=f"atm{h}", name=f"atm{h}_{c}")
                nc.vector.tensor_tensor(am[:], atp[:], mask_sb[:], Alu.mult)
                atm.append(am)

            if stage < 4:
                return
            # O psum (i, [attn_h | qk_h] x2): intra + inter contributions
            ops = []
            for h in range(HPC):
                oph = o_psum.tile([P, D + 1], f32, tag="o", name=f"o{h}_{c}")
                nc.tensor.matmul(
                    oph[:], atm[h][:], va[:, h * (D + 1) : (h + 1) * (D + 1)],
                    start=True, stop=(c == 0),
                )
                if c > 0:
                    nc.tensor.matmul(
                        oph[:], qp_t[st][h * D : (h + 1) * D, csl],
                        s_sb[c % 2][h * D : (h + 1) * D, :],
                        start=False, stop=True,
                    )
                ops.append(oph)

            if stage < 5:
                return
            # state update S += K'_s^T-outer  (col-packed pair), then copy for next chunk
            for h in range(HPC):
                nc.tensor.matmul(
                    s_ps[h * D : (h + 1) * D, :], ks[:, h * D : (h + 1) * D],
                    va[:, h * (D + 1) : (h + 1) * (D + 1)],
                    tile_position=(0, h * D),
                    start=(c == 0), stop=(c == nch - 1),
                    skip_group_check=True,
                )
            if c < nch - 1:
                nc.scalar.activation(s_sb[(c + 1) % 2][:], s_ps[:], AF.Copy)

            if stage < 6:
                return
            # normalize: recip of qk columns (64, 129), scale, transpose back
            rc = sc_pool.tile([P, HPC], f32, tag="rc")
            for h in range(HPC):
                nc.vector.reciprocal(rc[:, h : h + 1], ops[h][:, D : D + 1])
            osb = sc_pool.tile([P, P], f16, tag="osb")
            for h in range(HPC):
                nc.vector.tensor_scalar(
                    osb[:, h * D : (h + 1) * D], ops[h][:, 0:D],
                    rc[:, h : h + 1], None, Alu.mult,
                )
            otp = at_psum.tile([P, P], f16, tag="at", name=f"otp_{c}")
            nc.tensor.transpose(otp[:], osb[:], id_sb[:])
            ott = ot_pool.tile([P, P], f16, tag="ott")
            nc.scalar.activation(ott[:], otp[:], AF.Copy)

            if stage < 7:
                return
            # final projection for this chunk + store
            ob = out_pool.tile([P, D_MODEL], f16, tag="ob")
            fps0 = f_psum.tile([P, ST], f32, tag="f", name=f"f0_{c}")
            nc.tensor.matmul(fps0[:], ott[:], wc_sb[:, 0:ST], start=True, stop=True)
            fps1 = f_psum.tile([P, ST], f32, tag="f", name=f"f1_{c}")
            nc.tensor.matmul(fps1[:], ott[:], wc_sb[:, ST:D_MODEL], start=True, stop=True)
            if c % 2 == 0:
                nc.scalar.activation(ob[:, 0:ST], fps0[:], AF.Copy)
                nc.scalar.activation(ob[:, ST:D_MODEL], fps1[:], AF.Copy)
            else:
                nc.vector.tensor_copy(ob[:, 0:ST], fps0[:])
                nc.vector.tensor_copy(ob[:, ST:D_MODEL], fps1[:])
            nc.sync.dma_start(out[c * C : (c + 1) * C, :], ob[:])


        for st in range(nst):
            emit_stile(st)
            for c in range(4 * st, min(4 * st + 4, nch)):
                emit_chunk(c)

    nc.finalize()
    return nc


def _prep_inputs(v, k, q, wq_w, wq_b, wk_w, wk_b, wv_w, wv_b, wc_w, wc_b, wg):
    f16 = np.float16
    qT = np.ascontiguousarray(q[0].T).astype(f16)
    kT = np.ascontiguousarray(k[0].T).astype(f16)
    vT = np.ascontiguousarray(v[0].T).astype(f16)
    wg2 = np.concatenate([wg, wg], axis=0).astype(f16)  # (128, 64)
    ng2 = np.full((P, R), -0.5, f16)
    ident = np.eye(P, dtype=f16)
    mask = np.triu(np.ones((P, P), np.float32)).astype(f16)  # mask[j,i]=1 iff j<=i
    aux = np.concatenate([ident, mask, wg2, ng2], axis=1)  # (128, 384)
    in_maps = []
    for c in range(N_CORES):
        cs = slice(c * CW, (c + 1) * CW)
        bqkv = np.stack([
            (wq_b[cs] * NORM_D).astype(np.float32),
            (wk_b[cs] * NORM_D).astype(np.float32),
            wv_b[cs].astype(np.float32),
        ], axis=1)
        in_maps.append({
            "qT": qT, "kT": kT, "vT": vT,
            "wq": wq_w[:, cs].astype(f16),
            "wk": wk_w[:, cs].astype(f16),
            "wv": wv_w[:, cs].astype(f16),
            "bqkv": bqkv,
            "aux": aux,
            "wc": wc_w[cs, :].astype(f16),
        })
    return in_maps


def kernel(**inputs):
    from concourse.bass_utils import run_bass_kernel_spmd

    if "nc" not in _CACHE:
        _CACHE["nc"] = _build_bass()
    nc = _CACHE["nc"]
    in_maps = _prep_inputs(**inputs)
    res = run_bass_kernel_spmd(nc, in_maps, core_ids=list(range(N_CORES)))
    _CACHE["last_results"] = res
    acc = np.zeros((S, D_MODEL), np.float32)
    for c in range(N_CORES):
        acc += res.results[c]["out"].astype(np.float32)
    acc += inputs["wc_b"].astype(np.float32)[None, :]
    return acc[None, :, :]


if __name__ == "__main__":
    import reference

    inp = {k: np.asarray(v) for k, v in reference.setup_inputs().items()}
    got = kernel(**inp)
    print("kernel out", got.shape, got.dtype)
